# revision 1
# baseline (speedup 1.0000x reference)
"""Bass/Trainium2 kernel for DenseAtt: out = sigmoid(x@w_i [:,None] + x@w_j [None,:] + b).

Sharding: rows of the (8192, 8192) output are split across 8 NeuronCores
(1024 rows each). Every core receives the full x (needed for the column
projection b_full = x @ w_j) plus its local row block (for a_local = x_l @ w_i),
computes its row block of sigmoid(a_local[:,None] + b_full[None,:] + b), and the
host concatenates the row blocks.

Device-side plan (per core), seg-major over 4 column segments of 2048:
  1. DMA x in 512-row chunks, PE-transpose each 128x128 tile (identity
     matmul) into PSUM, DVE-copy to SBUF -> xT chunks [128 feat, 512 rows].
  2. matmul lhsT = w_j replicated across its free dim [128,128], rhs = xT
     chunk -> 4 chunks fill a 4-bank PSUM tile [128, 2048] where every
     partition holds b_full for those rows.
  3. a column (once, from the xl input): lhsT = xlT tile [128 feat, 128
     rows], rhs = w_i [128,1] -> PSUM [128,1] = proj_i for the local rows;
     linear bias b is folded in via a replicated column.
  4. Per segment, 8 sigmoid ACTs (one per local 128-row tile) read b_full
     DIRECTLY from PSUM with bias = the per-partition a column, each
     followed by a 1MB DMA store of [128, 2048] to the output row block.

The kernel is DMA-bound (32MB of output stores per core at ~360GB/s); the
projection prologue and sigmoids hide under the store stream. The cost-model
simulated exec is ~118us vs a ~106us pure-transfer bound.
"""

import numpy as np

_N = 8192          # rows/cols of the output
_D = 128           # feature dim
_M = 8             # cores
_R = _N // _M      # 1024 rows per core
_CH = 512          # rows per transpose chunk
_NCH = _N // _CH   # 16 chunks
_SEG = 2048        # output column segment width
_NSEG = _N // _SEG # 4 segments

_nc_cache = None


def _split_multi_waits(nc, mybir, max_keep=1):
    """Walrus on this toolchain only encodes ONE sem wait per instruction
    (NEURON_ISA_TPB_EVENTS has a single wait slot); Tile emits multi-wait
    sync_info. Split extras onto NoOps inserted right before the instruction
    on the same engine."""
    n_split = 0
    for fn in nc.m.functions:
        for bb in fn.blocks:
            newlist = []
            changed = False
            for inst in list(bb.instructions):
                si = inst.sync_info
                if si is not None and si.on_wait and len(si.on_wait) > max_keep:
                    waits = list(si.on_wait)
                    extra, keep = waits[:-max_keep], waits[-max_keep:]
                    for k, w in enumerate(extra):
                        newlist.append(
                            mybir.InstNoOp(
                                name=f"{inst.name}-waitsplit{k}",
                                engine=inst.engine,
                                sync_info=mybir.SyncInfo(on_wait=[w], on_update=[]),
                                bass_nofuse=True,
                            )
                        )
                        n_split += 1
                    inst.sync_info = mybir.SyncInfo(
                        on_wait=keep, on_update=list(si.on_update)
                    )
                    changed = True
                newlist.append(inst)
            if changed:
                bb.instructions = newlist
    return n_split


def _build():
    global _nc_cache
    if _nc_cache is not None:
        return _nc_cache

    import concourse.bass as bass
    import concourse.mybir as mybir
    from concourse.tile import TileContext

    f32 = mybir.dt.float32
    Sigmoid = mybir.ActivationFunctionType.Sigmoid
    Identity = mybir.ActivationFunctionType.Identity

    nc = bass.Bass("TRN2", debug=False, num_devices=_M)

    x_d = nc.dram_tensor("x", [_N, _D], f32, kind="ExternalInput")
    xl_d = nc.dram_tensor("xl", [_R, _D], f32, kind="ExternalInput")
    # packed constants: [:, :128] = eye(128), [:, 128] = w_i, [:, 129] = w_j,
    # [0, 130] = linear bias b
    cst_d = nc.dram_tensor("cst", [_D, _D + 3], f32, kind="ExternalInput")
    out_d = nc.dram_tensor("out", [_R, _N], f32, kind="ExternalOutput")

    # row index = t*128 + p  ->  [p, t, d] view for chunked partition loads
    xv = x_d.ap().rearrange("(t p) d -> p t d", p=128)    # [128, 64, 128]
    xlv = xl_d.ap().rearrange("(t p) d -> p t d", p=128)  # [128, 8, 128]

    with TileContext(nc) as tc:
        with (
            tc.tile_pool(name="const", bufs=1) as cpool,
            tc.tile_pool(name="xin", bufs=8) as xpool,
            tc.tile_pool(name="xt", bufs=4) as xtpool,
            tc.tile_pool(name="outp", bufs=8) as opool,
            tc.tile_pool(name="pt", bufs=2, space="PSUM") as pt_pool,
            tc.tile_pool(name="pb", bufs=1, space="PSUM") as pb_pool,
            tc.tile_pool(name="pa", bufs=2, space="PSUM") as pa_pool,
        ):
            cst_sb = cpool.tile([128, _D + 3], f32)
            nc.sync.dma_start(out=cst_sb[:], in_=cst_d[:])
            eye_sb = cst_sb[:, 0:_D]
            wi_sb = cst_sb[:, _D:_D + 1]
            wj_sb = cst_sb[:, _D + 1:_D + 2]
            b_sb = cst_sb[0:1, _D + 2:_D + 3]

            ones_sb = cpool.tile([1, 128], f32)
            nc.vector.memset(ones_sb[:], 1.0)
            zeros_sb = cpool.tile([128, 128], f32)
            nc.vector.memset(zeros_sb[:], 0.0)
            # w_j broadcast along free dim: wj_rep[k, m] = w_j[k] for all m
            wj_rep = cpool.tile([128, 128], f32)
            nc.vector.tensor_scalar_add(out=wj_rep[:], in0=zeros_sb[:], scalar1=wj_sb)

            # replicate linear bias across partitions: bcol[p] = b
            p_bc = pa_pool.tile([128, 1], f32, tag="pa")
            nc.tensor.matmul(p_bc[:], ones_sb[:], b_sb)
            bcol_sb = cpool.tile([128, 1], f32)
            nc.vector.tensor_copy(out=bcol_sb[:], in_=p_bc[:])

            # ---- local projection a = xl @ w_i (column layout [128, 8]) ----
            a_raw = cpool.tile([128, _R // 128], f32)
            for c in range(_R // _CH):  # 2 chunks of 512 local rows
                xl_sb = xpool.tile([128, _CH // 128, 128], f32, tag="xin")
                nc.sync.dma_start(out=xl_sb[:], in_=xlv[:, 4 * c:4 * c + 4, :])
                pt = pt_pool.tile([128, _CH], f32)
                for j in range(_CH // 128):
                    nc.tensor.transpose(
                        pt[:, j * 128:(j + 1) * 128], xl_sb[:, j, :], eye_sb
                    )
                xlT = xtpool.tile([128, _CH], f32, tag="xt")
                nc.vector.tensor_copy(out=xlT[:], in_=pt[:])
                for r in range(_CH // 128):
                    pa = pa_pool.tile([128, 1], f32, tag="pa")
                    nc.tensor.matmul(pa[:], xlT[:, r * 128:(r + 1) * 128], wi_sb)
                    rt = c * 4 + r
                    nc.vector.tensor_copy(out=a_raw[:, rt:rt + 1], in_=pa[:])
            a_sb = cpool.tile([128, _R // 128], f32)
            nc.vector.tensor_scalar_add(out=a_sb[:], in0=a_raw[:], scalar1=bcol_sb[:])

            # ---- seg-major main loop ----
            # For each 2048-wide column segment: matmul b_full into a 4-bank
            # PSUM tile (replicated across partitions), then 8 sigmoid ACTs
            # (one per local row-tile) read it DIRECTLY from PSUM with the
            # per-partition a column as bias, each followed by a 1MB store.
            for s in range(_NSEG):
                pb = pb_pool.tile([128, _SEG], f32, tag="pb")
                for q in range(_SEG // _CH):  # 4 chunks per segment
                    ch = (_SEG // _CH) * s + q
                    x_sb = xpool.tile([128, _CH // 128, 128], f32, tag="xin")
                    nc.sync.dma_start(out=x_sb[:], in_=xv[:, 4 * ch:4 * ch + 4, :])
                    pt = pt_pool.tile([128, _CH], f32)
                    for j in range(_CH // 128):
                        nc.tensor.transpose(
                            pt[:, j * 128:(j + 1) * 128], x_sb[:, j, :], eye_sb
                        )
                    xT = xtpool.tile([128, _CH], f32, tag="xt")
                    nc.vector.tensor_copy(out=xT[:], in_=pt[:])
                    nc.tensor.matmul(
                        pb[:, q * _CH:(q + 1) * _CH], wj_rep[:], xT[:]
                    )
                for rt in range(_R // 128):
                    o = opool.tile([128, _SEG], f32, tag="o")
                    nc.scalar.activation(
                        o[:], pb[:], Sigmoid, bias=a_sb[:, rt:rt + 1], scale=1.0,
                    )
                    nc.sync.dma_start(
                        out=out_d[rt * 128:(rt + 1) * 128, s * _SEG:(s + 1) * _SEG],
                        in_=o[:],
                    )

    _split_multi_waits(nc, mybir)

    _nc_cache = nc
    return nc


_runner_cache = None


def _get_runner(nc):
    """Build (once) a jitted shard_map callable around the bass_exec custom
    call, so repeated kernel() calls skip the per-call retrace/recompile that
    run_bass_kernel_spmd's fresh closures would incur."""
    global _runner_cache
    if _runner_cache is not None:
        return _runner_cache

    import jax
    from jax.experimental.shard_map import shard_map
    from jax.sharding import Mesh, PartitionSpec
    from concourse import bass2jax
    import concourse.mybir as mybir

    bass2jax.install_neuronx_cc_hook()

    in_names, out_names, out_avals, zero_outs = [], [], [], []
    for alloc in nc.m.functions[0].allocations:
        if not isinstance(alloc, mybir.MemoryLocationSet):
            continue
        name = alloc.memorylocations[0].name
        if alloc.kind == "ExternalInput":
            in_names.append(name)
        elif alloc.kind == "ExternalOutput":
            out_names.append(name)
            shape = tuple(alloc.tensor_shape)
            dtype = mybir.dt.np(alloc.dtype)
            out_avals.append(jax.core.ShapedArray(shape, dtype))
            zero_outs.append(np.zeros(shape, dtype))

    partition_name = nc.partition_id_tensor.name if nc.partition_id_tensor else None
    if partition_name is not None:
        in_names = [n for n in in_names if n != partition_name]
    n_params = len(in_names)
    all_names = in_names + out_names
    if partition_name is not None:
        all_names = all_names + [partition_name]

    def _body(*args):
        operands = list(args)
        if partition_name is not None:
            operands.append(bass2jax.partition_id_tensor())
        outs = bass2jax._bass_exec_p.bind(
            *operands,
            out_avals=tuple(out_avals),
            in_names=tuple(all_names),
            out_names=tuple(out_names),
            lowering_input_output_aliases=(),
            sim_require_finite=True,
            sim_require_nnan=True,
            nc=nc,
        )
        return tuple(outs)

    devices = jax.devices()[:_M]
    mesh = Mesh(np.asarray(devices), ("core",))
    nspecs = n_params + len(out_names)
    fn = jax.jit(
        shard_map(
            _body,
            mesh=mesh,
            in_specs=(PartitionSpec("core"),) * nspecs,
            out_specs=(PartitionSpec("core"),) * len(out_names),
            check_rep=False,
        ),
        keep_unused=True,
    )
    # Stage the (all-zero) output operands on device once; without donation
    # they are never consumed, so every call reuses them instead of shipping
    # 256MB of zeros through the relay each time.
    from jax.sharding import NamedSharding

    sh = NamedSharding(mesh, PartitionSpec("core"))
    zeros_dev = [
        jax.device_put(np.zeros((_M * z.shape[0], *z.shape[1:]), z.dtype), sh)
        for z in zero_outs
    ]
    _runner_cache = (fn, in_names, zeros_dev)
    return _runner_cache


class _Res:
    exec_time_ns = None
    results = None


def _make_in_maps(inputs):
    x = np.ascontiguousarray(np.asarray(inputs["x"], dtype=np.float32))
    w = np.asarray(inputs["w"], dtype=np.float32)
    b = np.asarray(inputs["b"], dtype=np.float32)
    assert x.shape == (_N, _D), x.shape

    cst = np.zeros((_D, _D + 3), dtype=np.float32)
    cst[:, :_D] = np.eye(_D, dtype=np.float32)
    cst[:, _D] = w[0, :_D]
    cst[:, _D + 1] = w[0, _D:]
    cst[0, _D + 2] = b[0]

    return [
        {
            "x": x,
            "xl": np.ascontiguousarray(x[c * _R:(c + 1) * _R]),
            "cst": cst,
        }
        for c in range(_M)
    ]


def _run(inputs, trace=False, trace_cores=None):
    from concourse._compat import axon_active

    nc = _build()
    in_maps = _make_in_maps(inputs)

    if axon_active() and not trace:
        fn, in_names, zeros_dev = _get_runner(nc)
        args = [
            np.concatenate([m[name] for m in in_maps], axis=0) for name in in_names
        ] + list(zeros_dev)
        out_cat = np.asarray(fn(*args)[0])
        return _Res(), out_cat.reshape(_M * _R, _N)

    from concourse.bass_utils import run_bass_kernel_spmd

    res = run_bass_kernel_spmd(
        nc, in_maps, core_ids=list(range(_M)), trace=trace, trace_cores=trace_cores
    )
    out = np.concatenate([r["out"] for r in res.results], axis=0)
    return res, out


def kernel(**inputs):
    _, out = _run(inputs)
    return out

